# revision 2
# baseline (speedup 1.0000x reference)
"""GQA kernel for Trainium2, 8 NeuronCores — fused-pipeline version.

Sharding: core c = b*4 + kv (b data-parallel over batch, kv tensor-parallel
over the 4 KV head groups; each core owns 4 Q heads + 1 KV head). Each core
computes a partial output x[b] @ Wq[:,kv] -> attention -> @ Wo[kv rows]; the
host sums the 4 partials per batch (the row-sharded-Wo all-reduce).

Device schedule (single fused tile scope, 4 time-steps of 512 positions):
  P1(0) P1(1) [A(0)|P1(2)] [A(1)|P1(3)] A(2) A(3)
where P1(s) projects K/V/Q for t in [512s, 512s+512), A(J) runs causal
attention + output projection for q in [512J, 512J+512) (needs steps 0..J),
and [A|P1] interleaves their work units so projection matmuls fill the PE
while attention's exp chains drain (keeps the PE p-state ramp at full clock).

All SBUF operands are bf16 (PSUM stays fp32): same PE throughput in the
cost model, 2x DVE on the rope chain, half the DMA bytes. RMSNorm over the
partition (head) dim via ones-matmul broadcast; rsqrt is the single-table
Abs_reciprocal_sqrt activation; Square/Copy work rides on the DVE so the
Act engine only alternates between the rsqrt and exp tables at work-unit
granularity (few table loads). RoPE: qr = half-swap via SP HWDGE DMA,
out = qn*cos + qr*sin' with rotate_half's sign folded into the sin table.
V is transposed to natural [t, hd] layout with DMA-transpose (no PE/PSUM).

Attention per (J, h): S^T blocks [k,q] via lhsT=KT slices; P = exp(S/sqrt(d))
(no max subtraction: |S| <= sqrt(d) after RMSNorm); diagonal blocks are
computed at sliced widths (512/384/256/128) with one [128,128] triangular
mask on the diagonal sub-block; softmax denominator rows via ones-matmul
accumulation; O^T accumulated over k blocks in PSUM; OT = op * (1/L) on DVE;
out tile = OT^T @ Wo drained via DVE copy and SWDGE DMA on the Pool queue
(keeps SP free for x/weight prefetch).
"""

import numpy as np

B, T, D = 2, 2048, 2048
NH, NKV, HD = 16, 4, 128
GQ = NH // NKV            # 4 q heads per kv head
HQ = GQ * HD              # 512 q-dim per core
ROPE_BASE = 500000.0
EPS = 1e-5
SCALE = 1.0 / np.sqrt(HD)
NS = 4                    # time steps
ST = T // NS              # 512
NDC = D // 128            # 16 contraction chunks

_cached = {}


def _interleave(a, b):
    out = []
    n = max(len(a), len(b))
    for i in range(n):
        if i < len(a):
            out.append(a[i])
        if i < len(b):
            out.append(b[i])
    return out


def _build_program():
    import concourse.bacc as bacc
    import concourse.mybir as mybir
    from concourse import tile
    from concourse.bass import ts

    f32 = mybir.dt.float32
    bf16 = mybir.dt.bfloat16
    AF = mybir.ActivationFunctionType

    nc = bacc.Bacc()

    xt = nc.dram_tensor("xt", [D, T], bf16, kind="ExternalInput")
    wq = nc.dram_tensor("wq", [128, NDC, HQ], bf16, kind="ExternalInput")
    wk = nc.dram_tensor("wk", [128, NDC, HD], bf16, kind="ExternalInput")
    wv = nc.dram_tensor("wv", [128, NDC, HD], bf16, kind="ExternalInput")
    wo = nc.dram_tensor("wo", [128, GQ, D], bf16, kind="ExternalInput")
    cosd = nc.dram_tensor("cos", [HD, T], bf16, kind="ExternalInput")
    sind = nc.dram_tensor("sin", [HD, T], bf16, kind="ExternalInput")
    wqcd = nc.dram_tensor("wqc", [HD, 1], f32, kind="ExternalInput")
    wkcd = nc.dram_tensor("wkc", [HD, 1], f32, kind="ExternalInput")
    wqed = nc.dram_tensor("wqe", [HD, 1], f32, kind="ExternalInput")
    wked = nc.dram_tensor("wke", [HD, 1], f32, kind="ExternalInput")
    trid = nc.dram_tensor("tri", [128, 128], bf16, kind="ExternalInput")
    onesd = nc.dram_tensor("ones", [128, 128], bf16, kind="ExternalInput")
    onesnd = nc.dram_tensor("onesn", [128, 128], bf16, kind="ExternalInput")
    outd = nc.dram_tensor("out", [T, D], bf16, kind="ExternalOutput")

    xtr = xt.rearrange("(c p) t -> p c t", p=128)

    with tile.TileContext(nc) as tc:
        with (
            tc.tile_pool(name="CONST", bufs=1) as C,
            tc.tile_pool(name="BIGA", bufs=1) as A,
            tc.tile_pool(name="X", bufs=2) as X,
            tc.tile_pool(name="ROPE", bufs=3) as R,
            tc.tile_pool(name="VT", bufs=2) as VT,
            tc.tile_pool(name="PP", bufs=6) as PP,
            tc.tile_pool(name="LRS", bufs=2) as LRS,
            tc.tile_pool(name="OST", bufs=2) as OST,
            tc.tile_pool(name="PROJ", bufs=2, space="PSUM") as PROJ,
            tc.tile_pool(name="SPP", bufs=2, space="PSUM") as SPP,
            tc.tile_pool(name="OPP", bufs=2, space="PSUM") as OPP,
            tc.tile_pool(name="LRX", bufs=2, space="PSUM") as LRX,
        ):
            # ---- persistent SBUF state ----
            QT = A.tile([128, GQ, T], bf16, tag="QT")
            KT = A.tile([128, T], bf16, tag="KT")
            Vn = A.tile([128, T // 128, HD], bf16, tag="Vn")
            OT = A.tile([128, GQ, T], bf16, tag="OT")

            wk_sb = C.tile([128, NDC, HD], bf16, tag="wk")
            wv_sb = C.tile([128, NDC, HD], bf16, tag="wv")
            wq_sb = C.tile([128, NDC, HQ], bf16, tag="wq")
            wo_sb = C.tile([128, GQ, D], bf16, tag="wo")
            cos_sb = C.tile([128, T], bf16, tag="cos")
            sin_sb = C.tile([128, T], bf16, tag="sin")
            tri_sb = C.tile([128, 128], bf16, tag="tri")
            ones_sb = C.tile([128, 128], bf16, tag="ones")
            onesn_sb = C.tile([128, 128], bf16, tag="onesn")
            wqc = C.tile([128, 1], f32, tag="wqc")
            wkc = C.tile([128, 1], f32, tag="wkc")
            wqe = C.tile([128, 1], f32, tag="wqe")
            wke = C.tile([128, 1], f32, tag="wke")

            xts = []
            for s in range(NS):
                xt_s = X.tile([128, NDC, ST], bf16, tag="xt", name=f"xt{s}")
                xts.append(xt_s)

            # initial DMAs ordered by first-use time: K needs wk + x0 chunks,
            # then the K rope needs onesn/cos/sin/scalars, V needs wv, the
            # Q heads need wq slices, and P1(1) needs xts[1]
            nc.sync.dma_start(out=wk_sb, in_=wk[:, :, :])
            nc.sync.dma_start(out=xts[0][:, ts(0, 4), :], in_=xtr[:, ts(0, 4), :ST])
            nc.sync.dma_start(out=xts[0][:, ts(1, 4), :], in_=xtr[:, ts(1, 4), :ST])
            nc.sync.dma_start(out=wv_sb, in_=wv[:, :, :])
            nc.sync.dma_start(out=xts[0][:, ts(2, 4), :], in_=xtr[:, ts(2, 4), :ST])
            nc.sync.dma_start(out=xts[0][:, ts(3, 4), :], in_=xtr[:, ts(3, 4), :ST])
            nc.sync.dma_start(out=onesn_sb, in_=onesnd[:, :])
            nc.sync.dma_start(out=cos_sb, in_=cosd[:, :])
            nc.sync.dma_start(out=sin_sb, in_=sind[:, :])
            nc.sync.dma_start(out=wqc, in_=wqcd[:, :])
            nc.sync.dma_start(out=wkc, in_=wkcd[:, :])
            nc.sync.dma_start(out=wqe, in_=wqed[:, :])
            nc.sync.dma_start(out=wke, in_=wked[:, :])
            for h in range(GQ):
                nc.sync.dma_start(out=wq_sb[:, :, ts(h, 128)],
                                  in_=wq[:, :, ts(h, 128)])
            nc.sync.dma_start(out=ones_sb, in_=onesd[:, :])
            nc.sync.dma_start(out=tri_sb, in_=trid[:, :])

            def normrope(pp, wc, we, sl, out_sl):
                """RMSNorm over the partition (head) dim + norm weight +
                RoPE, in the [hd, t] layout."""
                sq = R.tile([128, ST], bf16, tag="sq")
                nc.scalar.activation(sq, pp, AF.Square)
                l2 = LRX.tile([128, ST], f32, tag="lx")
                nc.tensor.matmul(l2, onesn_sb, sq, start=True, stop=True)
                rc = R.tile([128, ST], f32, tag="rc")
                nc.scalar.activation(rc, l2, AF.Abs_reciprocal_sqrt,
                                     scale=wc, bias=we)
                qn = R.tile([128, ST], bf16, tag="qn")
                nc.vector.tensor_mul(qn, pp, rc)
                qr = R.tile([128, ST], bf16, tag="qr")
                nc.sync.dma_start(out=qr[:64], in_=qn[64:])
                nc.sync.dma_start(out=qr[64:], in_=qn[:64])
                t1 = R.tile([128, ST], bf16, tag="t1")
                nc.vector.tensor_mul(t1, qn, cos_sb[:, sl])
                t2 = R.tile([128, ST], bf16, tag="t2")
                nc.vector.tensor_mul(t2, qr, sin_sb[:, sl])
                nc.vector.tensor_add(out_sl, t1, t2)

            def p1_units(s):
                sl = ts(s, ST)
                xt_t = xts[s]

                def grp_k():
                    if s + 1 < NS:
                        nc.sync.dma_start(out=xts[s + 1],
                                          in_=xtr[:, :, ts(s + 1, ST)])
                    if s == 1:
                        nc.sync.dma_start(out=wo_sb, in_=wo[:, :, :])
                    kp = PROJ.tile([128, ST], f32, tag="pp")
                    for c in range(NDC):
                        nc.tensor.matmul(kp, wk_sb[:, c, :], xt_t[:, c, :],
                                         start=(c == 0), stop=(c == NDC - 1))
                    normrope(kp, wkc, wke, sl, KT[:, sl])

                def grp_v():
                    vp = PROJ.tile([128, ST], f32, tag="pp")
                    for c in range(NDC):
                        nc.tensor.matmul(vp, wv_sb[:, c, :], xt_t[:, c, :],
                                         start=(c == 0), stop=(c == NDC - 1))
                    vt = VT.tile([128, ST], bf16, tag="vt")
                    nc.vector.tensor_scalar_mul(vt, vp, 1.0)
                    for i in range(ST // 128):
                        nc.sync.dma_start(out=Vn[:, 4 * s + i, :],
                                          in_=vt[:, ts(i, 128)], transpose=True)

                def grp_q(h):
                    qp = PROJ.tile([128, ST], f32, tag="pp")
                    for c in range(NDC):
                        nc.tensor.matmul(qp, wq_sb[:, c, ts(h, 128)],
                                         xt_t[:, c, :],
                                         start=(c == 0), stop=(c == NDC - 1))
                    normrope(qp, wqc, wqe, sl, QT[:, h, sl])

                return ([grp_k, grp_v]
                        + [(lambda h=h: grp_q(h)) for h in range(GQ)])

            def attn_units(J):
                qsl = ts(J, ST)
                # in the non-interleaved tail (J >= 2) the PROJ pool is free;
                # alternate sp tiles between SPP and PROJ for a depth-4
                # S-matmul software pipeline that hides the S->exp->L round
                # trip. During the interleaved phase keep depth 2 (SPP only).
                deep = (J >= 2)
                depth = 4 if deep else 2

                def head(h):
                    op = OPP.tile([128, ST], f32, tag="op")
                    lr = LRX.tile([128, ST], f32, tag="lx")
                    # block descriptors: (kb, colskip) — diag block r skips
                    # its first 128*r q columns
                    blocks = [(kb, 0) for kb in range(4 * J)]
                    blocks += [(4 * J + r, 128 * r) for r in range(4)]
                    n = len(blocks)
                    sps, Ps = [None] * n, [None] * n

                    def emit_s(i):
                        kb, skip = blocks[i]
                        w = ST - skip
                        pool = (SPP, PROJ)[i % 2] if deep else SPP
                        tag = "pp" if (deep and i % 2) else "sp"
                        sp = pool.tile([128, ST], f32, tag=tag)
                        nc.tensor.matmul(sp[:, :w], KT[:, ts(kb, 128)],
                                         QT[:, h, J * ST + skip:(J + 1) * ST],
                                         start=True, stop=True)
                        P = PP.tile([128, ST], bf16, tag="p")
                        nc.scalar.activation(P[:, :w], sp[:, :w], AF.Exp,
                                             scale=SCALE)
                        if skip or kb == 4 * J:  # diagonal block
                            nc.vector.tensor_mul(P[:, :128], P[:, :128], tri_sb)
                        sps[i], Ps[i] = sp, P

                    for i in range(min(depth, n)):
                        emit_s(i)
                    for i in range(n):
                        if i + depth < n:
                            emit_s(i + depth)
                        kb, skip = blocks[i]
                        w = ST - skip
                        first = (i == 0)
                        last = (i == n - 1)
                        nc.tensor.matmul(lr[:, skip:], ones_sb, Ps[i][:, :w],
                                         start=first, stop=last,
                                         skip_group_check=True)
                        nc.tensor.matmul(op[:, skip:], Vn[:, kb, :],
                                         Ps[i][:, :w], start=first, stop=last,
                                         skip_group_check=True)
                    rc2 = LRS.tile([128, ST], f32, tag="rc2")
                    nc.vector.reciprocal(rc2, lr)
                    nc.vector.tensor_mul(OT[:, h, qsl], op, rc2)

                def outproj(qt):
                    ost = None
                    for c in range(D // ST):
                        pool = (SPP, OPP)[c % 2]
                        oup = pool.tile([128, ST], f32, tag=("sp", "op")[c % 2])
                        for hc in range(GQ):
                            nc.tensor.matmul(oup, OT[:, hc, ts(qt, 128)],
                                             wo_sb[:, hc, ts(c, ST)],
                                             start=(hc == 0),
                                             stop=(hc == GQ - 1))
                        if c % 2 == 0:
                            ost = OST.tile([128, 2 * ST], bf16, tag="ost")
                        nc.vector.tensor_scalar_mul(
                            ost[:, (c % 2) * ST:(c % 2 + 1) * ST], oup, 1.0)
                        if c % 2 == 1:  # one DMA per chunk pair
                            nc.gpsimd.dma_start(
                                out=outd[qt * 128:(qt + 1) * 128,
                                         (c - 1) * ST:(c + 1) * ST],
                                in_=ost)

                return ([(lambda h=h: head(h)) for h in range(GQ)]
                        + [(lambda q=q: outproj(q)) for q in range(4 * J, 4 * J + 4)])

            a2 = attn_units(2)
            a3 = attn_units(3)
            units = (p1_units(0) + p1_units(1)
                     + _interleave(attn_units(0), p1_units(2))
                     + _interleave(attn_units(1), p1_units(3))
                     + a2[:4]                       # A(2) heads
                     + _interleave(a3[:4], a2[4:])  # A(3) heads | A(2) outproj
                     + a3[4:])                      # A(3) outproj
            for u in units:
                u()

    nc.finalize()
    return nc


def _host_consts():
    import ml_dtypes
    bf = ml_dtypes.bfloat16
    inv = 1.0 / (ROPE_BASE ** (np.arange(0, HD, 2, dtype=np.float64) / HD))
    freqs = np.outer(np.arange(T, dtype=np.float64), inv)
    emb = np.concatenate([freqs, freqs], axis=-1)          # [T, HD]
    cosT = np.ascontiguousarray(np.cos(emb).T.astype(bf))  # [HD, T]
    sinT = np.sin(emb).T.astype(np.float64)
    sinT[:64] *= -1.0  # fold rotate_half's sign: out = qn*cos + swap(qn)*sin'
    sinT = np.ascontiguousarray(sinT.astype(bf))
    tri = np.ascontiguousarray(
        (np.arange(128)[:, None] <= np.arange(128)[None, :]).astype(bf))
    ones = np.ones((128, 128), bf)
    onesn = np.full((128, 128), 1.0 / HD, bf)
    return cosT, sinT, tri, ones, onesn


def _rearr_w(Wslice):
    """[D, n] -> [128, D//128, n] with row d = (c*128 + p)."""
    import ml_dtypes
    d, n = Wslice.shape
    return np.ascontiguousarray(
        Wslice.reshape(d // 128, 128, n).transpose(1, 0, 2)
        .astype(ml_dtypes.bfloat16))


def kernel(x, Wq, Wk, Wv, Wo, q_norm_w, k_norm_w):
    import ml_dtypes
    from concourse.bass_utils import run_bass_kernel_spmd

    bf = ml_dtypes.bfloat16
    if "nc" not in _cached:
        _cached["nc"] = _build_program()
        _cached["consts"] = _host_consts()
    nc = _cached["nc"]
    cosT, sinT, tri, ones, onesn = _cached["consts"]

    x = np.asarray(x, np.float32)
    Wq = np.asarray(Wq, np.float32)
    Wk = np.asarray(Wk, np.float32)
    Wv = np.asarray(Wv, np.float32)
    Wo = np.asarray(Wo, np.float32)
    qwf = np.asarray(q_norm_w, np.float64).reshape(HD, 1)
    kwf = np.asarray(k_norm_w, np.float64).reshape(HD, 1)
    qw = np.ascontiguousarray((1.0 / qwf ** 2).astype(np.float32))
    kw = np.ascontiguousarray((1.0 / kwf ** 2).astype(np.float32))
    qwe = np.ascontiguousarray((EPS / qwf ** 2).astype(np.float32))
    kwe = np.ascontiguousarray((EPS / kwf ** 2).astype(np.float32))

    xTb = [np.ascontiguousarray(x[b].T.astype(bf)) for b in range(B)]
    wcache = {}
    in_maps = []
    for core in range(8):
        b, kv = divmod(core, NKV)
        if kv not in wcache:
            wcache[kv] = (
                _rearr_w(Wq[:, kv * HQ:(kv + 1) * HQ]),
                _rearr_w(Wk[:, kv * HD:(kv + 1) * HD]),
                _rearr_w(Wv[:, kv * HD:(kv + 1) * HD]),
                np.ascontiguousarray(
                    Wo[kv * HQ:(kv + 1) * HQ, :].reshape(GQ, 128, D)
                    .transpose(1, 0, 2).astype(bf)),
            )
        wqr, wkr, wvr, wor = wcache[kv]
        in_maps.append({
            "xt": xTb[b], "wq": wqr, "wk": wkr, "wv": wvr, "wo": wor,
            "cos": cosT, "sin": sinT, "wqc": qw, "wkc": kw,
            "wqe": qwe, "wke": kwe,
            "tri": tri, "ones": ones, "onesn": onesn,
        })
    res = run_bass_kernel_spmd(nc, in_maps, list(range(8)))
    out = np.zeros((B, T, D), np.float64)
    for core in range(8):
        b = core // NKV
        out[b] += res.results[core]["out"].astype(np.float64)
    return out.astype(np.float32)


# revision 3
# speedup vs baseline: 1.0390x; 1.0390x over previous
"""GQA kernel for Trainium2, 8 NeuronCores — fused-pipeline version.

Sharding: core c = b*4 + kv (b data-parallel over batch, kv tensor-parallel
over the 4 KV head groups; each core owns 4 Q heads + 1 KV head). Each core
computes a partial output x[b] @ Wq[:,kv] -> attention -> @ Wo[kv rows]; the
host sums the 4 partials per batch (the row-sharded-Wo all-reduce).

Device schedule (single fused tile scope, 4 time-steps of 512 positions):
  P1(0) P1(1) [A(0)|P1(2)] [A(1)|P1(3)] A(2) A(3)
where P1(s) projects K/V/Q for t in [512s, 512s+512), A(J) runs causal
attention + output projection for q in [512J, 512J+512) (needs steps 0..J),
and [A|P1] interleaves their work units so projection matmuls fill the PE
while attention's exp chains drain (keeps the PE p-state ramp at full clock).

All SBUF operands are bf16 (PSUM stays fp32): same PE throughput in the
cost model, 2x DVE on the rope chain, half the DMA bytes. RMSNorm over the
partition (head) dim via ones-matmul broadcast; rsqrt is the single-table
Abs_reciprocal_sqrt activation; Square/Copy work rides on the DVE so the
Act engine only alternates between the rsqrt and exp tables at work-unit
granularity (few table loads). RoPE: qr = half-swap via SP HWDGE DMA,
out = qn*cos + qr*sin' with rotate_half's sign folded into the sin table.
V is transposed to natural [t, hd] layout with DMA-transpose (no PE/PSUM).

Attention per (J, h): S^T blocks [k,q] via lhsT=KT slices; P = exp(S/sqrt(d))
(no max subtraction: |S| <= sqrt(d) after RMSNorm); diagonal blocks are
computed at sliced widths (512/384/256/128) with one [128,128] triangular
mask on the diagonal sub-block; softmax denominator rows via ones-matmul
accumulation; O^T accumulated over k blocks in PSUM; OT = op * (1/L) on DVE;
out tile = OT^T @ Wo drained via DVE copy and SWDGE DMA on the Pool queue
(keeps SP free for x/weight prefetch).
"""

import numpy as np

B, T, D = 2, 2048, 2048
NH, NKV, HD = 16, 4, 128
GQ = NH // NKV            # 4 q heads per kv head
HQ = GQ * HD              # 512 q-dim per core
ROPE_BASE = 500000.0
EPS = 1e-5
SCALE = 1.0 / np.sqrt(HD)
NS = 4                    # time steps
ST = T // NS              # 512
NDC = D // 128            # 16 contraction chunks

_cached = {}


def _interleave(a, b):
    out = []
    n = max(len(a), len(b))
    for i in range(n):
        if i < len(a):
            out.append(a[i])
        if i < len(b):
            out.append(b[i])
    return out


def _build_program():
    import concourse.bacc as bacc
    import concourse.mybir as mybir
    from concourse import tile
    from concourse.bass import ts

    f32 = mybir.dt.float32
    bf16 = mybir.dt.bfloat16
    AF = mybir.ActivationFunctionType

    nc = bacc.Bacc()

    fp8 = mybir.dt.float8e4
    DRM = mybir.MatmulPerfMode.DoubleRow

    xt = nc.dram_tensor("xt", [D, T], bf16, kind="ExternalInput")
    xh8 = nc.dram_tensor("xh8", [D, T], fp8, kind="ExternalInput")
    xl8 = nc.dram_tensor("xl8", [D, T], fp8, kind="ExternalInput")
    wq = nc.dram_tensor("wq", [128, NDC, HQ], fp8, kind="ExternalInput")
    wk = nc.dram_tensor("wk", [128, NDC, HD], fp8, kind="ExternalInput")
    wql = nc.dram_tensor("wql", [128, NDC, HQ], fp8, kind="ExternalInput")
    wkl = nc.dram_tensor("wkl", [128, NDC, HD], fp8, kind="ExternalInput")
    wv = nc.dram_tensor("wv", [128, NDC, HD], bf16, kind="ExternalInput")
    wo = nc.dram_tensor("wo", [128, GQ, D], bf16, kind="ExternalInput")
    cosd = nc.dram_tensor("cos", [HD, T], bf16, kind="ExternalInput")
    sind = nc.dram_tensor("sin", [HD, T], bf16, kind="ExternalInput")
    wqcd = nc.dram_tensor("wqc", [HD, 1], f32, kind="ExternalInput")
    wkcd = nc.dram_tensor("wkc", [HD, 1], f32, kind="ExternalInput")
    wqed = nc.dram_tensor("wqe", [HD, 1], f32, kind="ExternalInput")
    wked = nc.dram_tensor("wke", [HD, 1], f32, kind="ExternalInput")
    trid = nc.dram_tensor("tri", [128, 128], bf16, kind="ExternalInput")
    onesd = nc.dram_tensor("ones", [128, 128], bf16, kind="ExternalInput")
    onesnd = nc.dram_tensor("onesn", [128, 128], bf16, kind="ExternalInput")
    outd = nc.dram_tensor("out", [T, D], bf16, kind="ExternalOutput")

    xtr = xt.rearrange("(c p) t -> p c t", p=128)
    xh8r = xh8.rearrange("(c p) t -> p c t", p=128)
    xl8r = xl8.rearrange("(c p) t -> p c t", p=128)

    with tile.TileContext(nc) as tc:
        with (
            tc.tile_pool(name="CONST", bufs=1) as C,
            tc.tile_pool(name="BIGA", bufs=1) as A,
            tc.tile_pool(name="X", bufs=2) as X,
            tc.tile_pool(name="ROPE", bufs=3) as R,
            tc.tile_pool(name="VT", bufs=2) as VT,
            tc.tile_pool(name="PP", bufs=6) as PP,
            tc.tile_pool(name="LRS", bufs=2) as LRS,
            tc.tile_pool(name="OST", bufs=2) as OST,
            tc.tile_pool(name="PROJ", bufs=2, space="PSUM") as PROJ,
            tc.tile_pool(name="SPP", bufs=2, space="PSUM") as SPP,
            tc.tile_pool(name="OPP", bufs=2, space="PSUM") as OPP,
            tc.tile_pool(name="LRX", bufs=2, space="PSUM") as LRX,
        ):
            # ---- persistent SBUF state ----
            QT = A.tile([128, GQ, T], bf16, tag="QT")
            KT = A.tile([128, T], bf16, tag="KT")
            Vn = A.tile([128, T // 128, HD], bf16, tag="Vn")
            OT = A.tile([128, GQ, T], bf16, tag="OT")

            wk_sb = C.tile([128, NDC, HD], fp8, tag="wk")
            wv_sb = C.tile([128, NDC, HD], bf16, tag="wv")
            wq_sb = C.tile([128, NDC, HQ], fp8, tag="wq")
            wql_sb = C.tile([128, NDC, HQ], fp8, tag="wql")
            wkl_sb = C.tile([128, NDC, HD], fp8, tag="wkl")
            xl8_sb = C.tile([128, NDC, ST], fp8, tag="xl8")
            wo_sb = C.tile([128, GQ, D], bf16, tag="wo")
            cos_sb = C.tile([128, T], bf16, tag="cos")
            sin_sb = C.tile([128, T], bf16, tag="sin")
            tri_sb = C.tile([128, 128], bf16, tag="tri")
            ones_sb = C.tile([128, 128], bf16, tag="ones")
            onesn_sb = C.tile([128, 128], bf16, tag="onesn")
            wqc = C.tile([128, 1], f32, tag="wqc")
            wkc = C.tile([128, 1], f32, tag="wkc")
            wqe = C.tile([128, 1], f32, tag="wqe")
            wke = C.tile([128, 1], f32, tag="wke")

            xts, x8s = [], []
            for s in range(NS):
                xt_s = X.tile([128, NDC, ST], bf16, tag="xt", name=f"xt{s}")
                x8_s = X.tile([128, NDC, ST], fp8, tag="x8", name=f"x8{s}")
                xts.append(xt_s)
                x8s.append(x8_s)

            # initial DMAs ordered by first-use time: K needs wk + x0 chunks,
            # then the K rope needs onesn/cos/sin/scalars, V needs wv, the
            # Q heads need wq slices, and P1(1) needs xts[1]
            nc.sync.dma_start(out=wk_sb, in_=wk[:, :, :])
            nc.sync.dma_start(out=x8s[0], in_=xh8r[:, :, :ST])
            nc.sync.dma_start(out=xl8_sb, in_=xl8r[:, :, :ST])
            nc.sync.dma_start(out=wkl_sb, in_=wkl[:, :, :])
            nc.sync.dma_start(out=wql_sb, in_=wql[:, :, :])
            nc.sync.dma_start(out=xts[0][:, ts(0, 4), :], in_=xtr[:, ts(0, 4), :ST])
            nc.sync.dma_start(out=xts[0][:, ts(1, 4), :], in_=xtr[:, ts(1, 4), :ST])
            nc.sync.dma_start(out=wv_sb, in_=wv[:, :, :])
            nc.sync.dma_start(out=xts[0][:, ts(2, 4), :], in_=xtr[:, ts(2, 4), :ST])
            nc.sync.dma_start(out=xts[0][:, ts(3, 4), :], in_=xtr[:, ts(3, 4), :ST])
            nc.sync.dma_start(out=onesn_sb, in_=onesnd[:, :])
            nc.sync.dma_start(out=cos_sb, in_=cosd[:, :])
            nc.sync.dma_start(out=sin_sb, in_=sind[:, :])
            nc.sync.dma_start(out=wqc, in_=wqcd[:, :])
            nc.sync.dma_start(out=wkc, in_=wkcd[:, :])
            nc.sync.dma_start(out=wqe, in_=wqed[:, :])
            nc.sync.dma_start(out=wke, in_=wked[:, :])
            for h in range(GQ):
                nc.sync.dma_start(out=wq_sb[:, :, ts(h, 128)],
                                  in_=wq[:, :, ts(h, 128)])
            nc.sync.dma_start(out=ones_sb, in_=onesd[:, :])
            nc.sync.dma_start(out=tri_sb, in_=trid[:, :])

            def rope_a(pp):
                """Projection-PSUM epilogue: square + partition-sum; stage the
                raw projection and the column sum-of-squares to SBUF so PSUM
                frees early. rsqrt runs later, batched per step (keeps Act in
                one function table at a time)."""
                sq = R.tile([128, ST], bf16, tag="sq")
                nc.scalar.activation(sq, pp, AF.Square)
                pc = R.tile([128, ST], bf16, tag="pc", bufs=6)
                nc.vector.tensor_scalar_mul(pc, pp, 1.0)
                l2 = LRX.tile([128, ST], f32, tag="lx")
                nc.tensor.matmul(l2, onesn_sb, sq, start=True, stop=True)
                l2c = R.tile([128, ST], f32, tag="l2c", bufs=6)
                nc.vector.tensor_scalar_mul(l2c, l2, 1.0)
                return [pc, l2c]

            def rope_rc(st, wc, we):
                rc = R.tile([128, ST], f32, tag="rc", bufs=6)
                nc.scalar.activation(rc, st[1], AF.Abs_reciprocal_sqrt,
                                     scale=wc, bias=we)
                st.append(rc)

            def rope_c(st, sl, out_sl):
                pc, _, rc = st
                qn = R.tile([128, ST], bf16, tag="qn")
                nc.vector.tensor_mul(qn, pc, rc)
                qr = R.tile([128, ST], bf16, tag="qr")
                nc.sync.dma_start(out=qr[:64], in_=qn[64:])
                nc.sync.dma_start(out=qr[64:], in_=qn[:64])
                t1 = R.tile([128, ST], bf16, tag="t1")
                nc.vector.tensor_mul(t1, qn, cos_sb[:, sl])
                t2 = R.tile([128, ST], bf16, tag="t2")
                nc.vector.tensor_mul(t2, qr, sin_sb[:, sl])
                nc.vector.tensor_add(out_sl, t1, t2)

            def p1_units(s):
                sl = ts(s, ST)
                xt_t = xts[s]
                x8_t = x8s[s]
                st = {}

                def grp_k():
                    if s + 1 < NS:
                        nc.sync.dma_start(out=xts[s + 1],
                                          in_=xtr[:, :, ts(s + 1, ST)])
                        nc.sync.dma_start(out=x8s[s + 1],
                                          in_=xh8r[:, :, ts(s + 1, ST)])
                    if s == 1:
                        nc.sync.dma_start(out=wo_sb, in_=wo[:, :, :])
                    kp = PROJ.tile([128, ST], f32, tag="pp")
                    kt_terms = ([(wk_sb, x8_t), (wk_sb, xl8_sb),
                                 (wkl_sb, x8_t)] if s == 0
                                else [(wk_sb, x8_t)])
                    nt = len(kt_terms)
                    for i in range(NDC // 2):
                        for t, (wt, xt8) in enumerate(kt_terms):
                            nc.tensor.matmul(kp, wt[:, 2 * i:2 * i + 2, :],
                                             xt8[:, 2 * i:2 * i + 2, :],
                                             start=(i == 0 and t == 0),
                                             stop=(i == NDC // 2 - 1 and t == nt - 1),
                                             perf_mode=DRM)
                    st["k"] = rope_a(kp)

                def grp_v():
                    vp = PROJ.tile([128, ST], f32, tag="pp")
                    for c in range(NDC):
                        nc.tensor.matmul(vp, wv_sb[:, c, :], xt_t[:, c, :],
                                         start=(c == 0), stop=(c == NDC - 1))
                    vt = VT.tile([128, ST], bf16, tag="vt")
                    nc.vector.tensor_scalar_mul(vt, vp, 1.0)
                    for i in range(ST // 128):
                        nc.sync.dma_start(out=Vn[:, 4 * s + i, :],
                                          in_=vt[:, ts(i, 128)], transpose=True)

                def grp_q(h):
                    qp = PROJ.tile([128, ST], f32, tag="pp")
                    qt_terms = ([(wq_sb, x8_t), (wq_sb, xl8_sb),
                                 (wql_sb, x8_t)] if s == 0
                                else [(wq_sb, x8_t)])
                    nt = len(qt_terms)
                    for i in range(NDC // 2):
                        for t, (wt, xt8) in enumerate(qt_terms):
                            nc.tensor.matmul(
                                qp, wt[:, 2 * i:2 * i + 2, ts(h, 128)],
                                xt8[:, 2 * i:2 * i + 2, :],
                                start=(i == 0 and t == 0),
                                stop=(i == NDC // 2 - 1 and t == nt - 1),
                                perf_mode=DRM)
                    st[h] = rope_a(qp)

                def rc_batch():
                    rope_rc(st["k"], wkc, wke)
                    for h in range(GQ):
                        rope_rc(st[h], wqc, wqe)

                def rope_k():
                    rope_c(st["k"], sl, KT[:, sl])

                def rope_q(h):
                    rope_c(st[h], sl, QT[:, h, sl])

                return ([grp_k, grp_v]
                        + [(lambda h=h: grp_q(h)) for h in range(GQ)]
                        + [rc_batch, rope_k]
                        + [(lambda h=h: rope_q(h)) for h in range(GQ)])

            def attn_units(J):
                qsl = ts(J, ST)
                # in the non-interleaved tail (J >= 2) the PROJ pool is free;
                # alternate sp tiles between SPP and PROJ for a depth-4
                # S-matmul software pipeline that hides the S->exp->L round
                # trip. During the interleaved phase keep depth 2 (SPP only).
                deep = (J >= 2)
                depth = 4 if deep else 2

                def head(h):
                    op = OPP.tile([128, ST], f32, tag="op")
                    lr = LRX.tile([128, ST], f32, tag="lx")
                    # block descriptors: (kb, colskip) — diag block r skips
                    # its first 128*r q columns
                    blocks = [(kb, 0) for kb in range(4 * J)]
                    blocks += [(4 * J + r, 128 * r) for r in range(4)]
                    n = len(blocks)
                    sps, Ps = [None] * n, [None] * n

                    def emit_s(i):
                        kb, skip = blocks[i]
                        w = ST - skip
                        pool = (SPP, PROJ)[i % 2] if deep else SPP
                        tag = "pp" if (deep and i % 2) else "sp"
                        sp = pool.tile([128, ST], f32, tag=tag)
                        nc.tensor.matmul(sp[:, :w], KT[:, ts(kb, 128)],
                                         QT[:, h, J * ST + skip:(J + 1) * ST],
                                         start=True, stop=True)
                        P = PP.tile([128, ST], bf16, tag="p")
                        nc.scalar.activation(P[:, :w], sp[:, :w], AF.Exp,
                                             scale=SCALE)
                        if skip or kb == 4 * J:  # diagonal block
                            nc.vector.tensor_mul(P[:, :128], P[:, :128], tri_sb)
                        sps[i], Ps[i] = sp, P

                    for i in range(min(depth, n)):
                        emit_s(i)
                    for i in range(n):
                        if i + depth < n:
                            emit_s(i + depth)
                        kb, skip = blocks[i]
                        w = ST - skip
                        first = (i == 0)
                        last = (i == n - 1)
                        nc.tensor.matmul(lr[:, skip:], ones_sb, Ps[i][:, :w],
                                         start=first, stop=last,
                                         skip_group_check=True)
                        nc.tensor.matmul(op[:, skip:], Vn[:, kb, :],
                                         Ps[i][:, :w], start=first, stop=last,
                                         skip_group_check=True)
                    rc2 = LRS.tile([128, ST], f32, tag="rc2")
                    nc.vector.reciprocal(rc2, lr)
                    nc.vector.tensor_mul(OT[:, h, qsl], op, rc2)

                def outproj(qt):
                    ost = None
                    for c in range(D // ST):
                        pool = (SPP, OPP)[c % 2]
                        oup = pool.tile([128, ST], f32, tag=("sp", "op")[c % 2])
                        for hc in range(GQ):
                            nc.tensor.matmul(oup, OT[:, hc, ts(qt, 128)],
                                             wo_sb[:, hc, ts(c, ST)],
                                             start=(hc == 0),
                                             stop=(hc == GQ - 1))
                        if c % 2 == 0:
                            ost = OST.tile([128, 2 * ST], bf16, tag="ost")
                        nc.vector.tensor_scalar_mul(
                            ost[:, (c % 2) * ST:(c % 2 + 1) * ST], oup, 1.0)
                        if c % 2 == 1:  # one DMA per chunk pair
                            nc.gpsimd.dma_start(
                                out=outd[qt * 128:(qt + 1) * 128,
                                         (c - 1) * ST:(c + 1) * ST],
                                in_=ost)

                return ([(lambda h=h: head(h)) for h in range(GQ)]
                        + [(lambda q=q: outproj(q)) for q in range(4 * J, 4 * J + 4)])

            a2 = attn_units(2)
            a3 = attn_units(3)
            units = (p1_units(0) + p1_units(1)
                     + _interleave(attn_units(0), p1_units(2))
                     + _interleave(attn_units(1), p1_units(3))
                     + a2[:4]                       # A(2) heads
                     + _interleave(a3[:4], a2[4:])  # A(3) heads | A(2) outproj
                     + a3[4:])                      # A(3) outproj
            for u in units:
                u()

    nc.finalize()
    return nc


def _host_consts():
    import ml_dtypes
    bf = ml_dtypes.bfloat16
    inv = 1.0 / (ROPE_BASE ** (np.arange(0, HD, 2, dtype=np.float64) / HD))
    freqs = np.outer(np.arange(T, dtype=np.float64), inv)
    emb = np.concatenate([freqs, freqs], axis=-1)          # [T, HD]
    cosT = np.ascontiguousarray(np.cos(emb).T.astype(bf))  # [HD, T]
    sinT = np.sin(emb).T.astype(np.float64)
    sinT[:64] *= -1.0  # fold rotate_half's sign: out = qn*cos + swap(qn)*sin'
    sinT = np.ascontiguousarray(sinT.astype(bf))
    tri = np.ascontiguousarray(
        (np.arange(128)[:, None] <= np.arange(128)[None, :]).astype(bf))
    ones = np.ones((128, 128), bf)
    onesn = np.full((128, 128), 1.0 / HD, bf)
    return cosT, sinT, tri, ones, onesn


def _rearr_w8_res(Wslice):
    import ml_dtypes
    d, n = Wslice.shape
    hi = Wslice.reshape(d // 128, 128, n).transpose(1, 0, 2) \
        .astype(ml_dtypes.float8_e4m3)
    res = Wslice.reshape(d // 128, 128, n).transpose(1, 0, 2) \
        - hi.astype(np.float32)
    return np.ascontiguousarray(res.astype(ml_dtypes.float8_e4m3))


def _rearr_w8(Wslice):
    import ml_dtypes
    d, n = Wslice.shape
    return np.ascontiguousarray(
        Wslice.reshape(d // 128, 128, n).transpose(1, 0, 2)
        .astype(ml_dtypes.float8_e4m3))


def _rearr_w(Wslice):
    """[D, n] -> [128, D//128, n] with row d = (c*128 + p)."""
    import ml_dtypes
    d, n = Wslice.shape
    return np.ascontiguousarray(
        Wslice.reshape(d // 128, 128, n).transpose(1, 0, 2)
        .astype(ml_dtypes.bfloat16))


def kernel(x, Wq, Wk, Wv, Wo, q_norm_w, k_norm_w):
    import ml_dtypes
    from concourse.bass_utils import run_bass_kernel_spmd

    bf = ml_dtypes.bfloat16
    if "nc" not in _cached:
        _cached["nc"] = _build_program()
        _cached["consts"] = _host_consts()
    nc = _cached["nc"]
    cosT, sinT, tri, ones, onesn = _cached["consts"]

    x = np.asarray(x, np.float32)
    Wq = np.asarray(Wq, np.float32)
    Wk = np.asarray(Wk, np.float32)
    Wv = np.asarray(Wv, np.float32)
    Wo = np.asarray(Wo, np.float32)
    WSC = 64.0
    qwf = np.asarray(q_norm_w, np.float64).reshape(HD, 1)
    kwf = np.asarray(k_norm_w, np.float64).reshape(HD, 1)
    qw = np.ascontiguousarray((1.0 / qwf ** 2).astype(np.float32))
    kw = np.ascontiguousarray((1.0 / kwf ** 2).astype(np.float32))
    qwe = np.ascontiguousarray((WSC * WSC * EPS / qwf ** 2).astype(np.float32))
    kwe = np.ascontiguousarray((WSC * WSC * EPS / kwf ** 2).astype(np.float32))

    f8 = ml_dtypes.float8_e4m3
    xTb = [np.ascontiguousarray(x[b].T.astype(bf)) for b in range(B)]
    x8b = [np.ascontiguousarray(x[b].T.astype(f8)) for b in range(B)]
    xl8b = [np.ascontiguousarray(
        (x[b].T - x8b[b].astype(np.float32)).astype(f8)) for b in range(B)]
    wcache = {}
    in_maps = []
    for core in range(8):
        b, kv = divmod(core, NKV)
        if kv not in wcache:
            wcache[kv] = (
                _rearr_w8(Wq[:, kv * HQ:(kv + 1) * HQ] * WSC),
                _rearr_w8(Wk[:, kv * HD:(kv + 1) * HD] * WSC),
                _rearr_w8_res(Wq[:, kv * HQ:(kv + 1) * HQ] * WSC),
                _rearr_w8_res(Wk[:, kv * HD:(kv + 1) * HD] * WSC),
                _rearr_w(Wv[:, kv * HD:(kv + 1) * HD]),
                np.ascontiguousarray(
                    Wo[kv * HQ:(kv + 1) * HQ, :].reshape(GQ, 128, D)
                    .transpose(1, 0, 2).astype(bf)),
            )
        wqr, wkr, wqlr, wklr, wvr, wor = wcache[kv]
        in_maps.append({
            "xt": xTb[b], "xh8": x8b[b], "xl8": xl8b[b],
            "wq": wqr, "wk": wkr, "wql": wqlr, "wkl": wklr,
            "wv": wvr, "wo": wor,
            "cos": cosT, "sin": sinT, "wqc": qw, "wkc": kw,
            "wqe": qwe, "wke": kwe,
            "tri": tri, "ones": ones, "onesn": onesn,
        })
    res = run_bass_kernel_spmd(nc, in_maps, list(range(8)))
    out = np.zeros((B, T, D), np.float64)
    for core in range(8):
        b = core // NKV
        out[b] += res.results[core]["out"].astype(np.float64)
    return out.astype(np.float32)
